# revision 5
# baseline (speedup 1.0000x reference)
"""DotProductDistributionHead kernel v4 for Trainium2 (Bass/Tile), 8-core data-parallel.

    h = gelu(x @ W_mu + b_mu)            # (B, D) with erf gelu
    logits[b, n] = h[b] . emb_table[candidates[b, n]]   (+ mu_bias gather, zero here)

v1-layout (tokens on partitions, non-transpose dma_gather) with:
  - bf16 emb table (halves gather bytes; 256B descriptors)
  - per-row token cap per (block, pass); ragged overflow + the sparse tail
    pass computed on host (kills most of the max-over-rows padding)
  - bf16 broadcast-multiply + bf16-input tensor_reduce (f32 out), 2x DVE rate
  - deep gather pipeline (multi-queue, many buffers)

Sharding: x/candidates split along batch across 8 cores; W_mu / b_mu /
emb_table replicated per core.
"""

import os

import numpy as np

import concourse.bacc as bacc
import concourse.bass as bass
import concourse.tile as tile
from concourse import mybir
from concourse.bass_utils import run_bass_kernel_spmd

B, N, D, V = 4096, 200, 128, 100000
NCORES = 8
B_LOC = B // NCORES          # 512 batch rows per core
NBLK = B_LOC // 128          # 4 blocks of 128 rows
PASS_SIZE = 32768
NPASS = (V + PASS_SIZE - 1) // PASS_SIZE   # 4
CHUNK_T = int(os.environ.get("KERNEL_CHUNK_T", 32))  # token cols per gather
CAP = int(os.environ.get("KERNEL_CAP", 64))
P3HOST = os.environ.get("KERNEL_P3HOST", "1") == "1"
GBUFS = int(os.environ.get("KERNEL_GBUFS", 10))
NQ = int(os.environ.get("KERNEL_NQ", 4))

TRACE = False
LAST_RESULTS = None
ACT_FUNC = "Gelu"
REPEATS = 1

_f32 = mybir.dt.float32
_bf16 = mybir.dt.bfloat16
_i16 = mybir.dt.int16

CONST_COLS = D + B_LOC + D   # [W | xT | b_mu replicated]

_program_cache = {}


def _chunks(total):
    out = []
    t0 = 0
    while t0 < total:
        out.append((t0, min(CHUNK_T, total - t0)))
        t0 += CHUNK_T
    return out


def _kernel_body(tc, consts, gidx, emb, out, t_table, out_cols):
    nc = tc.nc
    gelu = getattr(mybir.ActivationFunctionType, ACT_FUNC)
    total_words = sum(t * 8 for row in t_table for t in row)

    with (
        tc.tile_pool(name="const", bufs=1) as cpool,
        tc.tile_pool(name="psum", bufs=2, space="PSUM") as ppool,
        tc.tile_pool(name="outs", bufs=2) as outpool,
        tc.tile_pool(name="gather", bufs=GBUFS) as gpool,
        tc.tile_pool(name="scratch", bufs=int(os.environ.get("KERNEL_SBUFS", 4))) as spool,
    ):
        c_sb = cpool.tile([128, CONST_COLS], _f32)
        nc.sync.dma_start(c_sb[:], consts[:, :])
        W_sb = c_sb[:, 0:D]
        xT_sb = c_sb[:, D : D + B_LOC]
        bias_sb = c_sb[:, D + B_LOC : D + B_LOC + D]

        gidx_sb = cpool.tile([128, total_words], _i16)
        nc.sync.dma_start(gidx_sb[:], gidx[:, :])

        # h[b, d] bf16 for all 512 local rows: block c at h_sb[:, c*D:(c+1)*D]
        h_sb = cpool.tile([128, NBLK * D], _bf16)
        for c in range(NBLK):
            ps = ppool.tile([128, D], _f32)
            nc.tensor.matmul(
                out=ps[:], lhsT=xT_sb[:, c * 128 : (c + 1) * 128], rhs=W_sb,
                start=True, stop=True,
            )
            nc.vector.tensor_tensor(
                out=ps[:], in0=ps[:], in1=bias_sb, op=mybir.AluOpType.add
            )
            nc.scalar.activation(out=h_sb[:, c * D : (c + 1) * D], in_=ps[:], func=gelu)

        qrr = 0
        for _rep in range(REPEATS):
          word_off = 0
          for c in range(NBLK):
            h_blk = h_sb[:, c * D : (c + 1) * D]
            t_tot = sum(t_table[c])
            logits_sb = outpool.tile([128, max(t_tot, 1)], _f32)
            col = 0
            for k in range(NPASS):
                T = t_table[c][k]
                if T == 0:
                    continue
                emb_k = emb[k * PASS_SIZE :, :]
                for t0, tc_len in _chunks(T):
                    num = 128 * tc_len
                    G = gpool.tile([128, CHUNK_T * D], _bf16, tag="gtile")
                    nc.gpsimd.dma_gather(
                        out_ap=G[:, : tc_len * D].rearrange("p (t d) -> p t d", d=D),
                        in_ap=emb_k,
                        idxs_ap=gidx_sb[:, word_off + t0 * 8 : word_off + (t0 + tc_len) * 8],
                        num_idxs=num,
                        num_idxs_reg=num,
                        elem_size=D,
                        single_packet=False,
                        queue_num=qrr % NQ,
                    )
                    qrr += 1
                    if os.environ.get("KERNEL_SKIP_COMPUTE"):
                        continue
                    G3 = G[:, : tc_len * D].rearrange("p (t d) -> p t d", d=D)
                    h_bc = h_blk.unsqueeze(1).to_broadcast([128, tc_len, D])
                    prod = spool.tile([128, CHUNK_T * D], _bf16, tag="ptile")
                    nc.vector.tensor_tensor(
                        out=prod[:, : tc_len * D].rearrange("p (t d) -> p t d", d=D),
                        in0=G3, in1=h_bc, op=mybir.AluOpType.mult,
                    )
                    nc.vector.tensor_reduce(
                        out=logits_sb[:, col + t0 : col + t0 + tc_len],
                        in_=prod[:, : tc_len * D].rearrange("p (t d) -> p t d", d=D),
                        axis=mybir.AxisListType.X,
                        op=mybir.AluOpType.add,
                    )
                word_off += T * 8
                col += T
            if not os.environ.get("KERNEL_SKIP_OUT"):
                nc.sync.dma_start(
                    out[:, out_cols[c] : out_cols[c] + t_tot], logits_sb[:, :t_tot]
                )


def _build_program(t_table, out_cols, total_out_cols):
    key = (tuple(tuple(r) for r in t_table), ACT_FUNC, CHUNK_T, REPEATS, GBUFS, NQ,
           os.environ.get("KERNEL_SBUFS"),
           bool(os.environ.get("KERNEL_SKIP_COMPUTE")), bool(os.environ.get("KERNEL_SKIP_OUT")))
    if key in _program_cache:
        return _program_cache[key]
    nc = bacc.Bacc(
        "TRN2",
        target_bir_lowering=False,
        debug=False,
        enable_asserts=False,
        num_devices=NCORES,
        num_swdge_queues=4,
    )
    total_words = sum(t * 8 for row in t_table for t in row)
    consts = nc.dram_tensor("consts", (128, CONST_COLS), _f32, kind="ExternalInput").ap()
    gidx = nc.dram_tensor("gidx", (128, total_words), _i16, kind="ExternalInput").ap()
    emb = nc.dram_tensor("emb", (V, D), _bf16, kind="ExternalInput").ap()
    out = nc.dram_tensor("out", (128, total_out_cols), _f32, kind="ExternalOutput").ap()
    with tile.TileContext(nc) as tc:
        _kernel_body(tc, consts, gidx, emb, out, t_table, out_cols)
    nc.finalize()
    _program_cache[key] = nc
    return nc


def _to_bf16(a):
    np_bf16 = mybir.dt.np(_bf16)
    return np.ascontiguousarray(np.asarray(a, dtype=np.float32).astype(np_bf16))


def prepare(x, candidates, W_mu, b_mu, mu_bias, emb_table):
    x = np.asarray(x, dtype=np.float32)
    candidates = np.asarray(candidates).astype(np.int64)
    W_mu = np.ascontiguousarray(np.asarray(W_mu, dtype=np.float32))
    b_mu = np.asarray(b_mu, dtype=np.float32)
    emb_bf16 = _to_bf16(emb_table)

    core_masks = []
    for core in range(NCORES):
        cl = candidates[core * B_LOC : (core + 1) * B_LOC]
        blocks = []
        for c in range(NBLK):
            blk = cl[c * 128 : (c + 1) * 128]
            row = []
            for k in range(NPASS):
                lo, hi = k * PASS_SIZE, min((k + 1) * PASS_SIZE, V)
                row.append((blk >= lo) & (blk < hi))
            blocks.append(row)
        core_masks.append(blocks)

    # capped T per (block, pass), shared across cores
    t_table = [[0] * NPASS for _ in range(NBLK)]
    for c in range(NBLK):
        for k in range(NPASS):
            if P3HOST and k == NPASS - 1:
                continue
            m = max(core_masks[core][c][k].sum(axis=1).max() for core in range(NCORES))
            t_table[c][k] = min(int(m), CAP)

    out_cols = []
    acc = 0
    for c in range(NBLK):
        out_cols.append(acc)
        acc += sum(t_table[c])
    total_out_cols = max(acc, 1)
    total_words = sum(t * 8 for row in t_table for t in row)

    gidx_tiles = []
    col_maps = []
    for core in range(NCORES):
        cl = candidates[core * B_LOC : (core + 1) * B_LOC]
        words = np.zeros((128, total_words), dtype=np.uint16)
        col_map = np.full((B_LOC, N), -1, dtype=np.int64)
        woff = 0
        for c in range(NBLK):
            blk = cl[c * 128 : (c + 1) * 128]
            col = 0
            for k in range(NPASS):
                T = t_table[c][k]
                if T == 0:
                    continue
                mask = core_masks[core][c][k]
                I = np.zeros((128, T), dtype=np.uint16)
                for p in range(128):
                    ns = np.nonzero(mask[p])[0][:T]
                    I[p, : len(ns)] = (blk[p, ns] - k * PASS_SIZE).astype(np.uint16)
                    # out row = partition p; block c lands at col offset out_cols[c]
                    col_map[c * 128 + p, ns] = out_cols[c] + col + np.arange(len(ns))
                vals_flat = I.T.ravel()
                wrapped = vals_flat.reshape(T * 8, 16).T
                words[:, woff : woff + T * 8] = np.tile(wrapped, (8, 1))
                woff += T * 8
                col += T
        gidx_tiles.append(words.view(np.int16))
        col_maps.append(col_map)

    nc = _build_program(t_table, out_cols, total_out_cols)

    bias_tile = np.broadcast_to(b_mu.reshape(1, D), (128, D))
    in_maps = []
    for core in range(NCORES):
        sl = slice(core * B_LOC, (core + 1) * B_LOC)
        consts = np.concatenate([W_mu, x[sl].T, bias_tile], axis=1)
        in_maps.append(
            {
                "consts": np.ascontiguousarray(consts, dtype=np.float32),
                "gidx": np.ascontiguousarray(gidx_tiles[core]),
                "emb": emb_bf16,
            }
        )
    return nc, in_maps, col_maps


def assemble(results, col_maps):
    logits = np.zeros((B, N), dtype=np.float32)
    for core in range(len(results)):
        out_core = results[core]["out"]  # [128, total_out_cols]
        cm = col_maps[core]              # [B_LOC, N] col index or -1
        for c in range(NBLK):
            rows = slice(c * 128, (c + 1) * 128)
            sub = np.take_along_axis(out_core, np.maximum(cm[rows], 0), axis=1)
            logits[core * B_LOC + c * 128 : core * B_LOC + (c + 1) * 128] = sub
    return logits


def _erf(v):
    """Abramowitz-Stegun 7.1.26 erf approximation (|err| < 1.5e-7)."""
    s = np.sign(v)
    a = np.abs(v)
    t = 1.0 / (1.0 + 0.3275911 * a)
    y = 1.0 - (((((1.061405429 * t - 1.453152027) * t) + 1.421413741) * t
                - 0.284496736) * t + 0.254829592) * t * np.exp(-a * a)
    return s * y


def _host_fill(logits, col_maps, x, candidates, W_mu, b_mu, emb_table):
    cm = np.concatenate(col_maps, axis=0)
    bb, nn = np.nonzero(cm < 0)
    if len(bb) == 0:
        return logits
    ub, inv = np.unique(bb, return_inverse=True)
    pre = x[ub] @ W_mu + b_mu[None, :]
    h = 0.5 * pre * (1.0 + _erf(pre / np.sqrt(2.0)))
    rows = emb_table[candidates[bb, nn]]
    logits[bb, nn] = np.einsum("td,td->t", h[inv], rows).astype(np.float32)
    return logits


def kernel(x, candidates, W_mu, b_mu, mu_bias, emb_table):
    global LAST_RESULTS
    candidates = np.asarray(candidates).astype(np.int64)
    mu_bias = np.asarray(mu_bias, dtype=np.float32)
    nc, in_maps, col_maps = prepare(x, candidates, W_mu, b_mu, mu_bias, emb_table)
    ncores_run = int(os.environ.get("KERNEL_CORES", NCORES))
    res = run_bass_kernel_spmd(
        nc, in_maps[:ncores_run], core_ids=list(range(ncores_run)), trace=TRACE
    )
    LAST_RESULTS = res
    logits = assemble(res.results, col_maps[:ncores_run])
    logits = _host_fill(
        logits, col_maps[:ncores_run],
        np.asarray(x, dtype=np.float32), candidates,
        np.asarray(W_mu, dtype=np.float32), np.asarray(b_mu, dtype=np.float32),
        np.asarray(emb_table, dtype=np.float32),
    )
    if np.any(mu_bias):
        logits = logits + mu_bias[candidates]
    return np.ascontiguousarray(logits.astype(np.float32))
